# revision 26
# baseline (speedup 1.0000x reference)
"""Trainium2 Bass kernel for nn_Contracter (e3nn tensor product + message passing).

  reference:  x2_scatter = segment_sum(x2, idxs, N); x2g = x2_scatter[idxs]
              out[e,u,k] = sum_ij x1[e,u,i] * x2g[e,u,j] * ww3j[u,i,j,k]

  Sharding: edges sorted by node; each core owns a contiguous node range and
  all its edges (segment-sum fully core-local, no collectives).  Per core:
    sweep 1: per 128-node block, one-hot (is_equal vs iota) matmuls
             accumulate table[n,(u,j)] over the block's edge chunks.
    mid:     PE-transpose table -> tableT;   Ctable[n,(u,k,i)] =
             sum_j table[n,(u,j)] * ww3j[u,i,j,k]  via block-diag matmuls.
    sweep 2: per edge chunk: CG = onehotT @ Ctable[block]  (PE gather),
             T = x1 (bcast over k) * CG  (DVE/GpSimd bf16), tree-reduce
             over i, DMA out.
  Matmul path all bf16 (one-hots exact); final accumulate in fp32.
"""
import sys
sys.path.insert(0, "/opt/trn_rl_repo")
import numpy as np
import ml_dtypes
import concourse.bass as bass
import concourse.bacc as bacc
import concourse.mybir as mybir
import concourse.tile as tile
from concourse import bass_utils
from concourse.masks import make_identity

P = 128
E = 100_000
N = 10_000
NCORES = 8
MUL, BD = 32, 9
DIM = MUL * BD            # 288
CDIM = MUL * BD * BD      # 2592
f32 = mybir.dt.float32
bf16 = mybir.dt.bfloat16
BF = ml_dtypes.bfloat16

UGROUPS = [(0, 11), (11, 22), (22, 32)]
UP = 22                   # product split: u < UP on DVE
UA = 10                   # add1 split: u < UA on DVE

_CACHE = {}


# ----------------------------------------------------------------- host prep
def _plan(idxs, n_nodes=N):
    order = np.argsort(idxs, kind="stable")
    deg = np.bincount(idxs, minlength=n_nodes)
    cum = np.concatenate([[0], np.cumsum(deg)])
    n_bounds = [0]
    for c in range(1, NCORES):
        n_bounds.append(int(np.searchsorted(cum, c * len(idxs) / NCORES)))
    n_bounds.append(n_nodes)
    cores = [dict(n_lo=n_bounds[c], n_hi=n_bounds[c + 1]) for c in range(NCORES)]
    NB = int(np.ceil(max(cr["n_hi"] - cr["n_lo"] for cr in cores) / P))
    CPB = np.zeros(NB, dtype=int)
    for cr in cores:
        n_lo, n_hi = cr["n_lo"], cr["n_hi"]
        for b in range(NB):
            blo, bhi = n_lo + b * P, min(n_lo + (b + 1) * P, n_hi)
            cnt = int(cum[bhi] - cum[blo]) if blo < n_hi else 0
            CPB[b] = max(CPB[b], (cnt + P - 1) // P)
    CPB = np.maximum(CPB, 1)
    return dict(order=order, cum=cum, cores=cores, NB=NB, CPB=CPB,
                E_pad=int(P * CPB.sum()))


def _core_arrays(plan, idxs, x1, x2):
    NB, CPB, E_pad = plan["NB"], plan["CPB"], plan["E_pad"]
    order, cum = plan["order"], plan["cum"]
    n_chunks = E_pad // P
    per_core = []
    for cr in plan["cores"]:
        n_lo, n_hi = cr["n_lo"], cr["n_hi"]
        x1s = np.zeros((E_pad, MUL * 10), BF)
        x2s = np.zeros((E_pad, DIM), BF)
        idxf = np.zeros(E_pad, np.float32)
        src = np.full(E_pad, -1, np.int64)
        pos = 0
        for b in range(NB):
            blo, bhi = n_lo + b * P, min(n_lo + (b + 1) * P, n_hi)
            se, ee = (int(cum[blo]), int(cum[bhi])) if blo < n_hi else (0, 0)
            sl = order[se:ee]
            cnt = ee - se
            x1s[pos:pos + cnt] = np.pad(
                x1[sl].reshape(cnt, MUL, BD), ((0, 0), (0, 0), (0, 1))
            ).reshape(cnt, MUL * 10).astype(BF)
            x2s[pos:pos + cnt] = x2[sl].astype(BF)
            idxf[pos:pos + cnt] = (idxs[sl] - blo).astype(np.float32)
            src[pos:pos + cnt] = sl
            pos += P * int(CPB[b])
        # idxfT[p, c] = idxf[c*128 + p]
        idxfT = np.ascontiguousarray(idxf.reshape(n_chunks, P).T)
        per_core.append(dict(x1s=x1s, x2s=x2s, idxfT=idxfT, src=src))
    return per_core


# Ctable global column layout: A=[(u,k,i0-3)] (1152) | B=[(u,k,i4-7)] (1152)
# | C=[(u,k,i8)] (288).  Per u-group the C-build matmuls emit the group's
# A/B/C slices; WW_g cols = [A_g (gu*36) | B_g (gu*36) | C_g (gu*9)].
A0, B0, C0 = 0, 1152, 2304


def _build_WW(w3j, weights):
    ww3j = np.einsum("up,pijk->uijk", weights, w3j).astype(np.float32)
    WW = np.zeros((DIM, 891), np.float32)
    for (u0, u1) in UGROUPS:
        gu = u1 - u0
        for u in range(u0, u1):
            blk = ww3j[u].transpose(1, 2, 0)          # [j, k, i]
            ul = u - u0
            WW[u * 9:(u + 1) * 9, ul * 36:(ul + 1) * 36] = \
                blk[:, :, 0:4].reshape(9, 36)
            WW[u * 9:(u + 1) * 9, gu * 36 + ul * 36:gu * 36 + (ul + 1) * 36] = \
                blk[:, :, 4:8].reshape(9, 36)
            WW[u * 9:(u + 1) * 9, gu * 72 + ul * 9:gu * 72 + (ul + 1) * 9] = \
                blk[:, :, 8].reshape(9, 9)
    return WW.astype(BF)


# ----------------------------------------------------------------- device
def _build_nc(NB, CPB, E_pad):
    NBN = NB * P
    n_chunks = E_pad // P
    nc = bacc.Bacc("TRN2", target_bir_lowering=False, debug=False,
                   num_devices=NCORES)
    d_x1 = nc.dram_tensor("x1s", [E_pad, MUL * 10], bf16, kind="ExternalInput")
    d_x2 = nc.dram_tensor("x2s", [E_pad, DIM], bf16, kind="ExternalInput")
    d_idxfT = nc.dram_tensor("idxfT", [P, n_chunks], f32, kind="ExternalInput")
    d_iota = nc.dram_tensor("iota", [P, P], bf16, kind="ExternalInput")
    d_WW = nc.dram_tensor("WW", [DIM, 891], bf16, kind="ExternalInput")
    d_out = nc.dram_tensor("out", [E_pad, DIM], bf16, kind="ExternalOutput")

    chunk_of = []
    for b in range(NB):
        chunk_of += [b] * int(CPB[b])

    with tile.TileContext(nc) as tc:
        with tc.tile_pool(name="persist", bufs=1) as pp:
            iota_t = pp.tile([P, P], bf16)
            nc.sync.dma_start(iota_t[:], d_iota[:])
            identb = pp.tile([P, P], bf16)
            make_identity(nc, identb[:])
            idxT = pp.tile([P, n_chunks], f32)
            nc.sync.dma_start(idxT[:], d_idxfT[:])
            WWt = []
            for gi, (u0, u1) in enumerate(UGROUPS):
                w = pp.tile([(u1 - u0) * 9, 891], bf16, tag=f"ww{gi}")
                nc.sync.dma_start(w[:], d_WW[u0 * 9:u1 * 9, :])
                WWt.append(w)
            tableT = {}
            for gi, (u0, u1) in enumerate(UGROUPS):
                for b in range(NB):
                    t = pp.tile([(u1 - u0) * 9, P], bf16, tag=f"tT{gi}_{b}")
                    tableT[(gi, b)] = t
            Ctab = []
            for b in range(NB):
                ct = pp.tile([P, CDIM], bf16, tag=f"ct{b}")
                Ctab.append(ct)

            ohTs = []
            for c in range(n_chunks):
                o = pp.tile([P, P], bf16, tag=f"ohT{c}")
                ohTs.append(o)

            # one fused pass: per block: seg-sum chunks (+ onehot transpose),
            # table transpose, Ctable build, then that block's sweep-2 chunks.
            with tc.tile_pool(name="wk", bufs=4) as wk, \
                 tc.tile_pool(name="wkb", bufs=3) as wkb, \
                 tc.tile_pool(name="wks", bufs=6) as wks, \
                 tc.tile_pool(name="pseg", bufs=1, space="PSUM") as pseg, \
                 tc.tile_pool(name="ptp", bufs=1, space="PSUM") as ptp, \
                 tc.tile_pool(name="pcg", bufs=1, space="PSUM") as pcg:
                ci = 0
                for b in range(NB):
                    nch = int(CPB[b])
                    # ---- sweep 1 for block b
                    seg = pseg.tile([P, 512], f32, tag="sg")
                    for k in range(nch):
                        c = ci + k
                        x2t = wk.tile([P, DIM], bf16, tag="x2")
                        nc.sync.dma_start(x2t[:], d_x2[c * P:(c + 1) * P, :])
                        oh = wk.tile([P, P], bf16, tag="oh")
                        nc.vector.tensor_scalar(
                            out=oh[:], in0=iota_t[:], scalar1=idxT[:, c:c + 1],
                            scalar2=None, op0=mybir.AluOpType.is_equal)
                        nc.tensor.matmul(seg[:, :DIM], lhsT=oh[:], rhs=x2t[:],
                                         start=(k == 0), stop=(k == nch - 1))
                        tpo = ptp.tile([P, 512], bf16, tag="tp")
                        nc.tensor.transpose(tpo[:, :P], oh[:], identb[:])
                        nc.scalar.copy(ohTs[c][:], tpo[:, :P])
                    ci += nch
                    tabs = wk.tile([P, DIM], bf16, tag="tab")
                    nc.scalar.copy(tabs[:], seg[:, :DIM])
                    for gi, (u0, u1) in enumerate(UGROUPS):
                        r = (u1 - u0) * 9
                        tp = ptp.tile([P, 512], bf16, tag="tp")
                        nc.tensor.transpose(tp[:r, :P], tabs[:, u0 * 9:u1 * 9],
                                            identb[:])
                        nc.scalar.copy(tableT[(gi, b)][:], tp[:r, :P])
                    # ---- Ctable build for block b (psum shared w/ transposes)
                    for gi, (u0, u1) in enumerate(UGROUPS):
                        gu = u1 - u0
                        spans = [(0, gu * 36, A0 + u0 * 36),
                                 (gu * 36, gu * 72, B0 + u0 * 36),
                                 (gu * 72, gu * 81, C0 + u0 * 9)]
                        for (n0, n1, dcol) in spans:
                            acc = ptp.tile([P, 512], f32, tag="tp")
                            nc.tensor.matmul(acc[:, :n1 - n0],
                                             lhsT=tableT[(gi, b)][:],
                                             rhs=WWt[gi][:, n0:n1],
                                             start=True, stop=True)
                            nc.scalar.copy(Ctab[b][:, dcol:dcol + n1 - n0],
                                           acc[:, :n1 - n0])
                    # ---- sweep 2 for block b
                    for c in range(ci - nch, ci):
                        x1b = wks.tile([P, MUL * 10], bf16, tag="x1b")
                        nc.sync.dma_start(x1b[:], d_x1[c * P:(c + 1) * P, :])
                        cgb = wkb.tile([P, CDIM], bf16, tag="cgb")
                        cg = pcg.tile([P, CDIM], f32, tag="cg")
                        for n0 in range(0, CDIM, 512):
                            n1 = min(n0 + 512, CDIM)
                            nc.tensor.matmul(cg[:, n0:n1], lhsT=ohTs[c][:],
                                             rhs=Ctab[b][:, n0:n1],
                                             start=True, stop=True)
                        nc.scalar.copy(cgb[:], cg[:])
                        TA = wkb.tile([P, 1152], bf16, tag="TA")
                        TB = wkb.tile([P, 1152], bf16, tag="TB")
                        TC = wks.tile([P, DIM], bf16, tag="TC")
                        x1b4 = x1b[:].rearrange("p (u k i) -> p u k i",
                                                u=MUL, k=1, i=10)
                        for eng, u_s, u_e in ((nc.vector, 0, MUL),):
                            nu = u_e - u_s
                            eng.tensor_tensor(
                                out=TA[:, u_s * 36:u_e * 36].rearrange(
                                    "p (u k i) -> p u k i", u=nu, k=BD),
                                in0=x1b4[:, u_s:u_e, :, 0:4].to_broadcast(
                                    [P, nu, BD, 4]),
                                in1=cgb[:, A0 + u_s * 36:A0 + u_e * 36].rearrange(
                                    "p (u k i) -> p u k i", u=nu, k=BD),
                                op=mybir.AluOpType.mult)
                            eng.tensor_tensor(
                                out=TB[:, u_s * 36:u_e * 36].rearrange(
                                    "p (u k i) -> p u k i", u=nu, k=BD),
                                in0=x1b4[:, u_s:u_e, :, 4:8].to_broadcast(
                                    [P, nu, BD, 4]),
                                in1=cgb[:, B0 + u_s * 36:B0 + u_e * 36].rearrange(
                                    "p (u k i) -> p u k i", u=nu, k=BD),
                                op=mybir.AluOpType.mult)
                        nc.gpsimd.tensor_tensor(
                            out=TC[:].rearrange("p (u k) -> p u k", u=MUL),
                            in0=x1b4[:, :, :, 8].to_broadcast([P, MUL, BD]),
                            in1=cgb[:, C0:C0 + DIM].rearrange(
                                "p (u k) -> p u k", u=MUL),
                            op=mybir.AluOpType.mult)
                        R4 = wkb.tile([P, 1152], bf16, tag="R4")
                        nc.gpsimd.tensor_tensor(out=R4[:], in0=TA[:],
                                                in1=TB[:],
                                                op=mybir.AluOpType.add)
                        R1 = wks.tile([P, DIM], bf16, tag="R1")
                        with nc.allow_low_precision(reason="bf16 i-reduce"):
                            nc.vector.tensor_reduce(
                                out=R1[:].rearrange("p (u k) -> p u k", u=MUL),
                                in_=R4[:].rearrange("p (u k i) -> p u k i",
                                                    u=MUL, k=BD),
                                axis=mybir.AxisListType.X,
                                op=mybir.AluOpType.add)
                        outt = wks.tile([P, DIM], bf16, tag="outt")
                        nc.vector.tensor_tensor(out=outt[:], in0=R1[:],
                                                in1=TC[:],
                                                op=mybir.AluOpType.add)
                        nc.sync.dma_start(d_out[c * P:(c + 1) * P, :], outt[:])
    nc.compile()
    return nc


# ----------------------------------------------------------------- entry
def kernel(x1, x2, idxs, scatter_dim_size, w3j, weights):
    x1 = np.asarray(x1, dtype=np.float32)
    x2 = np.asarray(x2, dtype=np.float32)
    idxs_np = np.asarray(idxs).astype(np.int64)
    w3j = np.asarray(w3j, dtype=np.float32)
    weights = np.asarray(weights, dtype=np.float32)

    plan = _plan(idxs_np, int(scatter_dim_size))
    per_core = _core_arrays(plan, idxs_np, x1, x2)
    WW = _build_WW(w3j, weights)
    iota = np.broadcast_to(np.arange(P, dtype=np.float32)[None, :],
                           (P, P)).astype(BF)

    key = (plan["NB"], tuple(plan["CPB"]), plan["E_pad"])
    if key not in _CACHE:
        _CACHE[key] = _build_nc(plan["NB"], plan["CPB"], plan["E_pad"])
    nc = _CACHE[key]

    in_maps = [{"x1s": pc["x1s"], "x2s": pc["x2s"], "idxfT": pc["idxfT"],
                "iota": iota, "WW": WW} for pc in per_core]
    res = None
    for attempt in range(3):
        try:
            res = bass_utils.run_bass_kernel_spmd(nc, in_maps,
                                                  core_ids=list(range(NCORES)))
            break
        except Exception:
            if attempt == 2:
                raise
            import time as _time
            _time.sleep(5)
    out = np.zeros((E, DIM), np.float32)
    for pc, r in zip(per_core, res.results):
        real = pc["src"] >= 0
        out[pc["src"][real]] = r["out"][real].astype(np.float32)
    return out.reshape(E, MUL, BD)


if __name__ == "__main__":
    sys.path.insert(0, "/root/problem")
    import reference as ref
    import jax
    with jax.default_device(jax.devices("cpu")[0]):
        inputs = {k: np.asarray(v) if hasattr(v, "shape") else v
                  for k, v in ref.setup_inputs().items()}
    got = kernel(**inputs)
    print("kernel done", got.shape)
